# revision 1
# baseline (speedup 1.0000x reference)
"""Trainium2 Bass kernel for nn_ContrastiveLoss (N=8192, D=1024, 751 ids).

loss = (1/N) * sum_ij [ same(i,j) & sim<1 -> (1-sim) ; diff(i,j) & sim>0.3 -> sim ]
with sim = X @ X.T.

Strategy (8 NeuronCores):
  * Host: sort rows by label (loss is permutation invariant). Same-label
    pairs then live within +-63 of the diagonal (max class count ~28).
  * sim is symmetric -> only the upper block-triangle is computed:
    16 row-blocks of 512 -> 136 block-pairs (a<=b), exactly 17 per core
    (core c takes block-rows c and 15-c). Off-diagonal pairs weigh 2x.
  * Per block-pair: fp16 matmuls into [128, 512] PSUM tiles (fp32
    accumulate). Unmasked sums need no label mask:
      sum_j sim*1[sim>0.3] = sum relu(sim-0.3) + 0.3*count(sim>0.3),
    ScalarE Relu activations with fused accum_out + VectorE is_gt
    tensor_scalar with fused accum_out per PSUM tile.
  * Band correction (256-wide windows around the diagonal): for
    same-label pairs subtract the neg term and add relu(1-sim), with a
    device-side label-equality mask. Band items are interleaved between
    main items to keep the PE busy.
  * Host: gather per-item partial sums, weight (1x diag / 2x off-diag),
    reduce in float64.
"""

import sys

for _p in ("/opt/trn_rl_repo",):
    if _p not in sys.path:
        sys.path.append(_p)

import numpy as np

import concourse.bass as bass
import concourse.mybir as mybir
import concourse.tile as tile
from concourse import bacc
from concourse.bass_utils import run_bass_kernel_spmd

N = 8192           # rows
D = 1024           # feature dim
NCORES = 8
B = 512            # triangle block size
NB = N // B        # 16 block-rows
NIT = 17           # items (block-pairs) per core
MS = B // 128      # m-subtiles per item = 4
KT = D // 128      # contraction chunks = 8
MT = (N // NCORES) // 128  # band row-tiles per core = 8
BW = 256           # band window width
IW = 2 * B         # packed item width (lhs 512 | rhs 512)
MARGIN = 0.3

# item pair groups for wide DMA streaming
GROUPS = [(g, min(2, NIT - g)) for g in range(0, NIT, 2)]

f16 = mybir.dt.float16
f32 = mybir.dt.float32

# output columns: per-item relu sums [0,17), per-item counts [17,34),
# band corr [34,42); padded to 48
C_R = 0
C_C = NIT * MS          # 68
C_B = 2 * NIT * MS      # 136
C_OUT = C_B + MT        # 144

_CACHE = {}


def _core_items(c):
    """Block-pair list for core c: rows c and 15-c of the triangle."""
    items = [(c, b) for b in range(c, NB)]
    items += [(NB - 1 - c, b) for b in range(NB - 1 - c, NB)]
    assert len(items) == NIT
    return items


def _build_program():
    nc = bacc.Bacc("TRN2", target_bir_lowering=False, debug=False,
                   num_devices=NCORES)

    itemd = nc.dram_tensor("items", [D, NIT * IW], f16, kind="ExternalInput")
    blhs = nc.dram_tensor("blhs", [D, MT * 128], f16, kind="ExternalInput")
    bwin = nc.dram_tensor("bwin", [D, MT * BW], f16, kind="ExternalInput")
    wlab = nc.dram_tensor("wlab", [128, MT * BW], f16, kind="ExternalInput")
    rlab = nc.dram_tensor("rlab", [128, MT], f32, kind="ExternalInput")
    outp = nc.dram_tensor("out", [128, C_OUT], f32, kind="ExternalOutput")

    item_t = itemd.rearrange("(k p) m -> k p m", p=128)
    blhs_t = blhs.rearrange("(k p) m -> k p m", p=128)
    bwin_t = bwin.rearrange("(k p) w -> k p w", p=128)

    Relu = mybir.ActivationFunctionType.Relu
    Op = mybir.AluOpType

    with tile.TileContext(nc) as tc:
        with (
            tc.tile_pool(name="persist", bufs=1) as persist,
            tc.tile_pool(name="grp", bufs=3) as grpp,
            tc.tile_pool(name="scr", bufs=3) as scr,
            tc.tile_pool(name="band", bufs=3) as bandp,
            tc.tile_pool(name="psum_m", bufs=6, space="PSUM") as psum_m,
            tc.tile_pool(name="psum_b", bufs=2, space="PSUM") as psum_b,
        ):
            # ---- persistent band tiles (DMAs issued after group 0) ----
            blhs_sb = []
            bwin_sb = []
            for k in range(KT):
                tb = persist.tile([128, MT * BW], f16, name=f"bwin{k}")
                bwin_sb.append(tb)
                tl = persist.tile([128, MT * 128], f16, name=f"blhs{k}")
                blhs_sb.append(tl)
            wlab_sb = persist.tile([128, MT * BW], f16, name="wlab")
            rlab_sb = persist.tile([128, MT], f32, name="rlab")

            def band_loads(half):
                ks = range(0, KT // 2) if half == 0 else range(KT // 2, KT)
                for k in ks:
                    nc.sync.dma_start(bwin_sb[k][:], bwin_t[k])
                    nc.sync.dma_start(blhs_sb[k][:], blhs_t[k])
                if half == 1:
                    nc.sync.dma_start(wlab_sb[:], wlab[:])
                    nc.sync.dma_start(rlab_sb[:], rlab[:])

            stats = persist.tile([128, C_OUT], f32, name="stats")
            bias_m = persist.tile([128, 1], f32, name="bias_m")
            nc.vector.memset(bias_m[:], -MARGIN)

            def band_item(j):
                """One [128 x 256] diagonal-window correction."""
                ps = psum_b.tile([128, BW], f32, name="bb")
                pj = ps[:, :BW]
                for k in range(KT):
                    nc.tensor.matmul(
                        pj,
                        blhs_sb[k][:, j * 128:(j + 1) * 128],
                        bwin_sb[k][:, j * BW:(j + 1) * BW],
                        start=(k == 0), stop=(k == KT - 1),
                    )
                pos = bandp.tile([128, BW], f32, name="pos")
                rb = bandp.tile([128, BW], f32, name="rb")
                gt = bandp.tile([128, BW], f32, name="gt")
                # pos = relu(1 - s);  rb = relu(s - 0.3);  gt = 1[s > 0.3]
                nc.scalar.activation(pos[:], pj, Relu, bias=1.0, scale=-1.0)
                nc.scalar.activation(rb[:], pj, Relu, bias=bias_m[:])
                nc.vector.tensor_scalar(gt[:], pj, MARGIN, None, op0=Op.is_gt)
                # neg = rb + 0.3*gt ; corr = eq * (pos - neg)
                a = bandp.tile([128, BW], f32, name="a")
                nc.vector.scalar_tensor_tensor(
                    a[:], gt[:], MARGIN, pos[:], op0=Op.mult, op1=Op.subtract)
                b = bandp.tile([128, BW], f32, name="b")
                nc.vector.tensor_tensor(b[:], a[:], rb[:], op=Op.add)
                # b = neg - pos
                eq = bandp.tile([128, BW], f32, name="eq")
                nc.vector.tensor_scalar(
                    eq[:], wlab_sb[:, j * BW:(j + 1) * BW],
                    rlab_sb[:, j:j + 1], None, op0=Op.is_equal)
                crr = bandp.tile([128, BW], f32, name="crr")
                nc.vector.scalar_tensor_tensor(
                    crr[:], b[:], -1.0, eq[:], op0=Op.mult, op1=Op.mult,
                    accum_out=stats[:, C_B + j:C_B + j + 1])

            # ---- triangle sweep; band items interleaved after item 9+ ----
            nband = 0
            for g0, gw in GROUPS:
                gq = []
                for k in range(KT):
                    tg = grpp.tile([128, 2 * IW], f16, name=f"gq{k}")
                    nc.sync.dma_start(
                        tg[:, :gw * IW],
                        item_t[k, :, g0 * IW:(g0 + gw) * IW])
                    gq.append(tg)
                if g0 == 2:
                    band_loads(0)
                elif g0 == 4:
                    band_loads(1)
                for ii in range(gw):
                    it = g0 + ii
                    off = ii * IW
                    for m in range(MS):
                        ps = psum_m.tile([128, B], f32, name="mm")
                        for k in range(KT):
                            nc.tensor.matmul(
                                ps[:],
                                gq[k][:, off + m * 128:off + (m + 1) * 128],
                                gq[k][:, off + B:off + IW],
                                start=(k == 0), stop=(k == KT - 1),
                            )
                        col = it * MS + m
                        sr = scr.tile([128, B], f16, name="sr")
                        nc.scalar.activation(
                            sr[:], ps[:], Relu, bias=bias_m[:],
                            accum_out=stats[:, C_R + col:C_R + col + 1])
                        sc = scr.tile([128, B], f16, name="sc")
                        nc.vector.tensor_scalar(
                            sc[:], ps[:], MARGIN, None, op0=Op.is_gt,
                            op1=Op.add,
                            accum_out=stats[:, C_C + col:C_C + col + 1])
                    if it >= 8 and nband < MT:
                        band_item(nband)
                        nband += 1
            while nband < MT:
                band_item(nband)
                nband += 1

            nc.sync.dma_start(outp[:], stats[:])

    nc.compile()
    return nc


def _prepare_in_maps(X, t):
    perm = np.argsort(t, kind="stable")
    Xs = X[perm]
    ts = t[perm]
    counts = np.bincount(ts.astype(np.int64))
    maxc = int(counts.max()) if counts.size else 0
    assert maxc <= 64, f"class count {maxc} exceeds band half-width 64"
    XT = np.ascontiguousarray(Xs.T).astype(np.float16)  # [D, N]
    tsf = ts.astype(np.float16)                         # exact for ids < 2048

    in_maps = []
    weights = []
    for c in range(NCORES):
        items = _core_items(c)
        itemp = np.empty((D, NIT * IW), np.float16)
        w = np.empty(NIT, np.float64)
        for i, (a, b) in enumerate(items):
            itemp[:, i * IW:i * IW + B] = XT[:, a * B:(a + 1) * B]
            itemp[:, i * IW + B:(i + 1) * IW] = XT[:, b * B:(b + 1) * B]
            w[i] = 1.0 if a == b else 2.0
        weights.append(w)

        r0 = c * (N // NCORES)
        blhs = np.ascontiguousarray(XT[:, r0:r0 + MT * 128])
        bwin = np.empty((D, MT * BW), np.float16)
        wlaba = np.empty((128, MT * BW), np.float16)
        rlab = np.empty((128, MT), np.float32)
        for j in range(MT):
            p = r0 + j * 128
            w0 = min(max(p - 64, 0), N - BW)
            bwin[:, j * BW:(j + 1) * BW] = XT[:, w0:w0 + BW]
            wlaba[:, j * BW:(j + 1) * BW] = tsf[w0:w0 + BW][None, :]
            rlab[:, j] = ts[p:p + 128].astype(np.float32)
        in_maps.append({
            "items": itemp, "blhs": blhs, "bwin": bwin,
            "wlab": wlaba, "rlab": rlab,
        })
    return in_maps, weights


def _reduce_outputs(results, weights):
    tot = 0.0
    for c in range(NCORES):
        o = np.asarray(results[c]["out"], np.float64)
        r_items = o[:, C_R:C_C].sum(axis=0).reshape(NIT, MS).sum(axis=1)
        c_items = o[:, C_C:C_B].sum(axis=0).reshape(NIT, MS).sum(axis=1)
        neg_items = r_items + MARGIN * c_items
        tot += float((weights[c] * neg_items).sum())
        tot += float(o[:, C_B:C_B + MT].sum())
    return np.float32(tot / float(N))


def kernel(inputs, targets, _trace=False, _tmpdir=None):
    X = np.asarray(inputs, dtype=np.float32)
    t = np.asarray(targets)
    assert X.shape == (N, D)

    if "nc" not in _CACHE:
        _CACHE["nc"] = _build_program()
    nc = _CACHE["nc"]

    in_maps, weights = _prepare_in_maps(X, t)
    res = run_bass_kernel_spmd(
        nc, in_maps, list(range(NCORES)), trace=_trace, tmpdir=_tmpdir)
    loss = _reduce_outputs(res.results, weights)
    if _trace:
        return loss, res
    return loss



# revision 2
# speedup vs baseline: 1.8213x; 1.8213x over previous
"""Trainium2 Bass kernel for nn_ContrastiveLoss (N=8192, D=1024, 751 ids).

loss = (1/N) * sum_ij [ same(i,j) & sim<1 -> (1-sim) ; diff(i,j) & sim>0.3 -> sim ]
with sim = X @ X.T.

Strategy (8 NeuronCores):
  * Host: sort rows by label (loss is permutation invariant); same-label
    pairs then live within +-63 of the diagonal (max class count <= 64).
    Quantize X to fp8 e4m3 -> PE runs DoubleRow matmuls (K=256/pass).
  * sim is symmetric: orient each unordered block pair {a,b} of the 16
    512-row blocks toward head a if (b-a) mod 16 in 1..8 (a<8) / 1..7.
    Core c takes heads c (9 items) and c+8 (8 items) -> 17 items/core,
    all 136 pairs covered once. Off-diagonal pairs weigh 2x.
  * Slot-uniform program: core c's DRAM holds the 16 blocks rotated by
    c (slot s = block (c+s) mod 16), so one program serves all cores.
    Each block is DMA'd once (fp8, 512KB) and stays resident in SBUF;
    items read lhs from slot 0/8, rhs from slot i -> ~8.5MB DMA/core.
  * Per item: 4 [128,512] PSUM tiles, 4 DoubleRow matmuls each.
    Unmasked sums need no label mask:
      sum_j sim*1[sim>0.3] = sum relu(sim-0.3) + 0.3*count(sim>0.3),
    ScalarE Relu with fused accum_out + VectorE is_gt with accum_out.
  * Band correction (circular 256-wide windows around the diagonal) on
    rows of blocks c and c+8 (slots 0/8): for same-label pairs subtract
    the neg term and add relu(1-sim), label-equality masked.  Windows
    slice the *same* resident tiles, so the sim values cancel bitwise.
    Wrapped window columns are killed by the label mask.
  * Host: gather per-item partial sums, weight (1x diag / 2x off-diag),
    reduce in float64.
"""

import sys

for _p in ("/opt/trn_rl_repo",):
    if _p not in sys.path:
        sys.path.append(_p)

import numpy as np
import ml_dtypes

import concourse.bass as bass
import concourse.mybir as mybir
import concourse.tile as tile
from concourse import bacc
from concourse.bass_utils import run_bass_kernel_spmd

N = 8192           # rows
D = 1024           # feature dim
NCORES = 8
B = 512            # block size
NB = N // B        # 16 blocks/slots
NIT = 17           # items (block pairs) per core
MS = B // 128      # m-subtiles per item = 4
KT2 = D // 256     # DoubleRow contraction chunks = 4
MT = 8             # band row-tiles per core (2 groups x 4)
BW = 256           # band window width
MARGIN = 0.3

f8 = mybir.dt.float8e4
f16 = mybir.dt.float16
f32 = mybir.dt.float32

# output columns: per-subtile relu sums [0,68), counts [68,136),
# band corr [136,144)
C_R = 0
C_C = NIT * MS          # 68
C_B = 2 * NIT * MS      # 136
C_OUT = C_B + MT        # 144

_CACHE = {}


def _band_segments(gi, j):
    """Window segments (slot, col0, width) for band row-tile j of group gi.

    Window = circular [S*512 + j*128 - 64, +256) in slot-column space.
    """
    S = 8 * gi
    if j == 0:
        return [((S - 1) % NB, 448, 64), (S, 0, 192)]
    if j == 3:
        return [(S, 320, 192), ((S + 1) % NB, 0, 64)]
    return [(S, j * 128 - 64, 256)]


def _build_program():
    nc = bacc.Bacc("TRN2", target_bir_lowering=False, debug=False,
                   num_devices=NCORES)

    # [slot, partition, (g,h), col]: slot s holds block (c+s)%16 of X^T,
    # k-row (2g+h)*128+p, quantized to fp8 e4m3.
    xq = nc.dram_tensor("xq", [NB, 128, 2 * KT2, B], f8, kind="ExternalInput")
    wlab = nc.dram_tensor("wlab", [128, MT * BW], f16, kind="ExternalInput")
    rlab = nc.dram_tensor("rlab", [128, MT], f32, kind="ExternalInput")
    outp = nc.dram_tensor("out", [128, C_OUT], f32, kind="ExternalOutput")

    Relu = mybir.ActivationFunctionType.Relu
    Op = mybir.AluOpType
    DR = mybir.MatmulPerfMode.DoubleRow

    with tile.TileContext(nc) as tc:
        with (
            tc.tile_pool(name="persist", bufs=1) as persist,
            tc.tile_pool(name="scr", bufs=3) as scr,
            tc.tile_pool(name="band", bufs=3) as bandp,
            tc.tile_pool(name="psum_m", bufs=6, space="PSUM") as psum_m,
            tc.tile_pool(name="psum_b", bufs=2, space="PSUM") as psum_b,
        ):
            T = [persist.tile([128, 2 * KT2, B], f8, name=f"blk{s}")
                 for s in range(NB)]
            wlab_sb = persist.tile([128, MT * BW], f16, name="wlab")
            rlab_sb = persist.tile([128, MT], f32, name="rlab")
            stats = persist.tile([128, C_OUT], f32, name="stats")
            bias_m = persist.tile([128, 1], f32, name="bias_m")
            nc.vector.memset(bias_m[:], -MARGIN)

            def main_item(i):
                L = T[0] if i <= 8 else T[8]
                R = T[i] if i <= 8 else T[i - 1]
                for m in range(MS):
                    ps = psum_m.tile([128, B], f32, name="mm")
                    for g in range(KT2):
                        nc.tensor.matmul(
                            ps[:],
                            L[:, 2 * g:2 * g + 2, m * 128:(m + 1) * 128],
                            R[:, 2 * g:2 * g + 2, :],
                            start=(g == 0), stop=(g == KT2 - 1),
                            perf_mode=DR,
                        )
                    col = i * MS + m
                    sr = scr.tile([128, B], f16, name="sr")
                    nc.scalar.activation(
                        sr[:], ps[:], Relu, bias=bias_m[:],
                        accum_out=stats[:, C_R + col:C_R + col + 1])
                    sc = scr.tile([128, B], f16, name="sc")
                    nc.vector.tensor_scalar(
                        sc[:], ps[:], MARGIN, None, op0=Op.is_gt,
                        op1=Op.add,
                        accum_out=stats[:, C_C + col:C_C + col + 1])

            def band_item(gi, j):
                """One [128 x 256] diagonal-window correction."""
                S = 8 * gi
                bj = gi * 4 + j
                ps = psum_b.tile([128, BW], f32, name="bb")
                off = 0
                for (sl, c0, w) in _band_segments(gi, j):
                    pj = ps[:, off:off + w]
                    for g in range(KT2):
                        nc.tensor.matmul(
                            pj,
                            T[S][:, 2 * g:2 * g + 2, j * 128:(j + 1) * 128],
                            T[sl][:, 2 * g:2 * g + 2, c0:c0 + w],
                            start=(g == 0), stop=(g == KT2 - 1),
                            perf_mode=DR,
                        )
                    off += w
                pos = bandp.tile([128, BW], f32, name="pos")
                rb = bandp.tile([128, BW], f32, name="rb")
                gt = bandp.tile([128, BW], f32, name="gt")
                # pos = relu(1 - s);  rb = relu(s - 0.3);  gt = 1[s > 0.3]
                nc.scalar.activation(pos[:], ps[:], Relu, bias=1.0, scale=-1.0)
                nc.scalar.activation(rb[:], ps[:], Relu, bias=bias_m[:])
                nc.vector.tensor_scalar(gt[:], ps[:], MARGIN, None,
                                        op0=Op.is_gt)
                # neg = rb + 0.3*gt ; corr = eq * (pos - neg)
                a = bandp.tile([128, BW], f32, name="a")
                nc.vector.scalar_tensor_tensor(
                    a[:], gt[:], MARGIN, pos[:], op0=Op.mult,
                    op1=Op.subtract)
                b = bandp.tile([128, BW], f32, name="b")
                nc.vector.tensor_tensor(b[:], a[:], rb[:], op=Op.add)
                # b = neg - pos
                eq = bandp.tile([128, BW], f32, name="eq")
                nc.vector.tensor_scalar(
                    eq[:], wlab_sb[:, bj * BW:(bj + 1) * BW],
                    rlab_sb[:, bj:bj + 1], None, op0=Op.is_equal)
                crr = bandp.tile([128, BW], f32, name="crr")
                nc.vector.scalar_tensor_tensor(
                    crr[:], b[:], -1.0, eq[:], op0=Op.mult, op1=Op.mult,
                    accum_out=stats[:, C_B + bj:C_B + bj + 1])

            # band items interleaved where their slots are resident
            bands_after = {
                1: [(0, 1)], 2: [(0, 2)], 3: [(0, 3)],
                10: [(1, 1)], 11: [(1, 2)], 12: [(1, 3)], 13: [(1, 0)],
                16: [(0, 0)],
            }

            nc.sync.dma_start(T[0][:], xq[0])
            nc.sync.dma_start(T[1][:], xq[1])
            nc.sync.dma_start(wlab_sb[:], wlab[:])
            nc.sync.dma_start(rlab_sb[:], rlab[:])
            for i in range(NIT):
                if i + 2 < NB:
                    nc.sync.dma_start(T[i + 2][:], xq[i + 2])
                main_item(i)
                for (gi, j) in bands_after.get(i, ()):
                    band_item(gi, j)

            nc.sync.dma_start(outp[:], stats[:])

    nc.compile()
    return nc


def _prepare_in_maps(X, t):
    perm = np.argsort(t, kind="stable")
    Xs = X[perm]
    ts = t[perm].astype(np.int64)
    counts = np.bincount(ts)
    maxc = int(counts.max()) if counts.size else 0
    assert maxc <= 64, f"class count {maxc} exceeds band half-width 64"
    XT = np.ascontiguousarray(Xs.astype(ml_dtypes.float8_e4m3).T)  # [D, N]
    # [b, p, (g,h), col]
    blocks = np.ascontiguousarray(
        XT.reshape(KT2, 2, 128, NB, B).transpose(3, 2, 0, 1, 4)
    ).reshape(NB, 128, 2 * KT2, B)
    tsf = ts.astype(np.float16)  # exact for ids < 2048

    in_maps = []
    for c in range(NCORES):
        order = [(c + s) % NB for s in range(NB)]
        xqc = np.ascontiguousarray(blocks[order])
        wl = np.empty((128, MT * BW), np.float16)
        rl = np.empty((128, MT), np.float32)
        for gi in range(2):
            base = (c + 8 * gi) % NB
            for j in range(4):
                bj = gi * 4 + j
                r0 = base * B + j * 128
                idx = (np.arange(BW) + r0 - 64) % N
                wl[:, bj * BW:(bj + 1) * BW] = tsf[idx][None, :]
                rl[:, bj] = ts[r0:r0 + 128].astype(np.float32)
        in_maps.append({"xq": xqc, "wlab": wl, "rlab": rl})
    return in_maps


# item weights: diag items (slots 0 and 8 vs themselves) 1x, rest 2x
_W_ITEM = np.array([1.0] + [2.0] * 8 + [1.0] + [2.0] * 7, np.float64)


def _reduce_outputs(results):
    tot = 0.0
    for c in range(NCORES):
        o = np.asarray(results[c]["out"], np.float64)
        r_items = o[:, C_R:C_C].sum(axis=0).reshape(NIT, MS).sum(axis=1)
        c_items = o[:, C_C:C_B].sum(axis=0).reshape(NIT, MS).sum(axis=1)
        neg_items = r_items + MARGIN * c_items
        tot += float((_W_ITEM * neg_items).sum())
        tot += float(o[:, C_B:C_B + MT].sum())
    return np.float32(tot / float(N))


def kernel(inputs, targets, _trace=False, _tmpdir=None):
    X = np.asarray(inputs, dtype=np.float32)
    t = np.asarray(targets)
    assert X.shape == (N, D)

    if "nc" not in _CACHE:
        _CACHE["nc"] = _build_program()
    nc = _CACHE["nc"]

    in_maps = _prepare_in_maps(X, t)
    res = run_bass_kernel_spmd(
        nc, in_maps, list(range(NCORES)), trace=_trace, tmpdir=_tmpdir)
    loss = _reduce_outputs(res.results)
    if _trace:
        return loss, res
    return loss


# revision 9
# speedup vs baseline: 1.8609x; 1.0217x over previous
"""Trainium2 Bass kernel for nn_ContrastiveLoss (N=8192, D=1024, 751 ids).

loss = (1/N) * sum_ij [ same(i,j) & sim<1 -> (1-sim) ; diff(i,j) & sim>0.3 -> sim ]
with sim = X @ X.T.

Strategy (8 NeuronCores):
  * Host: sort rows by label (loss is permutation invariant); same-label
    pairs then live within +-63 of the diagonal (max class count <= 64).
    Quantize X to fp8 e4m3 -> PE runs DoubleRow matmuls (K=256/pass).
  * sim is symmetric: orient each unordered block pair {a,b} of the 16
    512-row blocks toward head a if (b-a) mod 16 in 1..8 (a<8) / 1..7.
    Core c takes heads c (9 items) and c+8 (8 items) -> 17 items/core,
    all 136 pairs covered once. Off-diagonal pairs weigh 2x.
  * Slot-uniform program: core c's DRAM holds the 16 blocks rotated by
    c (slot s = block (c+s) mod 16), so one program serves all cores.
    Each block is DMA'd once (fp8, 512KB) and stays resident in SBUF;
    items read lhs from slot 0/8, rhs from slot i -> ~8.5MB DMA/core.
  * Per item: 4 [128,512] PSUM tiles, 4 DoubleRow matmuls each.
    Unmasked sums need no label mask:
      sum_j sim*1[sim>0.3] = sum relu(sim-0.3) + 0.3*count(sim>0.3),
    ScalarE Relu with fused accum_out + VectorE is_gt with accum_out.
  * Band correction (circular 256-wide windows around the diagonal) on
    rows of blocks c and c+8 (slots 0/8): for same-label pairs subtract
    the neg term and add relu(1-sim), label-equality masked.  Windows
    slice the *same* resident tiles, so the sim values cancel bitwise.
    Wrapped window columns are killed by the label mask.
  * Host: gather per-item partial sums, weight (1x diag / 2x off-diag),
    reduce in float64.
"""

import sys

for _p in ("/opt/trn_rl_repo",):
    if _p not in sys.path:
        sys.path.append(_p)

import numpy as np
import ml_dtypes

import concourse.bass as bass
import concourse.mybir as mybir
import concourse.tile as tile
from concourse import bacc
from concourse.bass_utils import run_bass_kernel_spmd

N = 8192           # rows
D = 1024           # feature dim
NCORES = 8
B = 512            # block size
NB = N // B        # 16 blocks/slots
NIT = 17           # items (block pairs) per core
MS = B // 128      # m-subtiles per item = 4
KT2 = D // 256     # DoubleRow contraction chunks = 4
MT = 8             # band row-tiles per core (2 groups x 4)
BW = 256           # band window width
MARGIN = 0.3

f8 = mybir.dt.float8e4
f16 = mybir.dt.float16
f32 = mybir.dt.float32

# output columns: per-half-item relu sums [0,34), counts [34,68),
# band corr [68,76), warmup junk [76]
MH = 2                  # m-subtile pairs per item ([128,1024] PSUM tiles)
C_R = 0
C_C = NIT * MH          # 34
C_B = 2 * NIT * MH      # 68
C_OUT = C_B + MT + 1    # 77

_CACHE = {}


def _band_segments(gi, j):
    """Window segments (slot, col0, width) for band row-tile j of group gi.

    Window = circular [S*512 + j*128 - 64, +256) in slot-column space.
    """
    S = 8 * gi
    if j == 0:
        return [((S - 1) % NB, 448, 64), (S, 0, 192)]
    if j == 3:
        return [(S, 320, 192), ((S + 1) % NB, 0, 64)]
    return [(S, j * 128 - 64, 256)]


def _build_program():
    nc = bacc.Bacc("TRN2", target_bir_lowering=False, debug=False,
                   num_devices=NCORES)

    # [slot, partition, (g,h), col]: slot s holds block (c+s)%16 of X^T,
    # k-row (2g+h)*128+p, quantized to fp8 e4m3.
    xq = nc.dram_tensor("xq", [NB, 128, 2 * KT2, B], f8, kind="ExternalInput")
    wlab = nc.dram_tensor("wlab", [128, MT * BW], f16, kind="ExternalInput")
    rlab = nc.dram_tensor("rlab", [128, MT], f32, kind="ExternalInput")
    outp = nc.dram_tensor("out", [128, C_OUT], f32, kind="ExternalOutput")

    Relu = mybir.ActivationFunctionType.Relu
    Op = mybir.AluOpType
    DR = mybir.MatmulPerfMode.DoubleRow

    with tile.TileContext(nc) as tc:
        with (
            tc.tile_pool(name="persist", bufs=1) as persist,
            tc.tile_pool(name="scr", bufs=3) as scr,
            tc.tile_pool(name="band", bufs=3) as bandp,
            tc.tile_pool(name="psum_m", bufs=3, space="PSUM") as psum_m,
            tc.tile_pool(name="psum_b", bufs=2, space="PSUM") as psum_b,
        ):
            T = [persist.tile([128, 2 * KT2, B], f8, name=f"blk{s}")
                 for s in range(NB)]
            wlab_sb = persist.tile([128, MT * BW], f16, name="wlab")
            rlab_sb = persist.tile([128, MT], f32, name="rlab")
            stats = persist.tile([128, C_OUT], f32, name="stats")
            bias_m = persist.tile([128, 1], f32, name="bias_m")
            nc.vector.memset(bias_m[:], -MARGIN)

            # HAM warmup: junk matmuls keep the PE busy while the first
            # block DMA lands, so the clock gate opens before real work.
            wm = persist.tile([128, 2, B], f8, name="wm")
            nc.vector.memset(wm[:], 0.125)
            wps = psum_m.tile([128, 2 * B], f32, name="mm")
            for w in range(6):
                nc.tensor.matmul(
                    wps[:, :B], wm[:, :, :128], wm[:],
                    start=(w == 0), stop=(w == 5), perf_mode=DR)
            wsr = scr.tile([128, B], f16, name="wsr")
            nc.scalar.activation(
                wsr[:], wps[:, :B], Relu,
                accum_out=stats[:, C_OUT - 1:C_OUT])

            def main_item(i):
                L = T[0] if i <= 8 else T[8]
                R = T[i] if i <= 8 else T[i - 1]
                for h in range(MH):
                    ps = psum_m.tile([128, 2 * B], f32, name="mm")
                    for m2 in range(2):
                        m = 2 * h + m2
                        pj = ps[:, m2 * B:(m2 + 1) * B]
                        for g in range(KT2):
                            nc.tensor.matmul(
                                pj,
                                L[:, 2 * g:2 * g + 2, m * 128:(m + 1) * 128],
                                R[:, 2 * g:2 * g + 2, :],
                                start=(g == 0), stop=(g == KT2 - 1),
                                perf_mode=DR,
                            )
                    col = i * MH + h
                    sr = scr.tile([128, 2 * B], f16, name="sr")
                    nc.scalar.activation(
                        sr[:], ps[:], Relu, bias=bias_m[:],
                        accum_out=stats[:, C_R + col:C_R + col + 1])
                    sc = scr.tile([128, 2 * B], f16, name="sc")
                    nc.vector.tensor_scalar(
                        sc[:], ps[:], MARGIN, None, op0=Op.is_gt,
                        op1=Op.add,
                        accum_out=stats[:, C_C + col:C_C + col + 1])

            def band_item(gi, j):
                """One [128 x 256] diagonal-window correction."""
                S = 8 * gi
                bj = gi * 4 + j
                ps = psum_b.tile([128, BW], f32, name="bb")
                off = 0
                for (sl, c0, w) in _band_segments(gi, j):
                    pj = ps[:, off:off + w]
                    for g in range(KT2):
                        nc.tensor.matmul(
                            pj,
                            T[S][:, 2 * g:2 * g + 2, j * 128:(j + 1) * 128],
                            T[sl][:, 2 * g:2 * g + 2, c0:c0 + w],
                            start=(g == 0), stop=(g == KT2 - 1),
                            perf_mode=DR,
                        )
                    off += w
                pos = bandp.tile([128, BW], f32, name="pos")
                rb = bandp.tile([128, BW], f32, name="rb")
                gt = bandp.tile([128, BW], f32, name="gt")
                # pos = relu(1 - s);  rb = relu(s - 0.3);  gt = 1[s > 0.3]
                nc.scalar.activation(pos[:], ps[:], Relu, bias=1.0, scale=-1.0)
                nc.scalar.activation(rb[:], ps[:], Relu, bias=bias_m[:])
                nc.vector.tensor_scalar(gt[:], ps[:], MARGIN, None,
                                        op0=Op.is_gt)
                # neg = rb + 0.3*gt ; corr = eq * (pos - neg)
                a = bandp.tile([128, BW], f32, name="a")
                nc.vector.scalar_tensor_tensor(
                    a[:], gt[:], MARGIN, pos[:], op0=Op.mult,
                    op1=Op.subtract)
                b = bandp.tile([128, BW], f32, name="b")
                nc.vector.tensor_tensor(b[:], a[:], rb[:], op=Op.add)
                # b = neg - pos
                eq = bandp.tile([128, BW], f32, name="eq")
                nc.vector.tensor_scalar(
                    eq[:], wlab_sb[:, bj * BW:(bj + 1) * BW],
                    rlab_sb[:, bj:bj + 1], None, op0=Op.is_equal)
                crr = bandp.tile([128, BW], f32, name="crr")
                nc.vector.scalar_tensor_tensor(
                    crr[:], b[:], -1.0, eq[:], op0=Op.mult, op1=Op.mult,
                    accum_out=stats[:, C_B + bj:C_B + bj + 1])

            # band items interleaved where their slots are resident
            bands_after = {
                1: [(0, 1)], 2: [(0, 2)], 3: [(0, 3)],
                10: [(1, 1)], 11: [(1, 2)], 12: [(1, 3)], 13: [(1, 0)],
                16: [(0, 0)],
            }

            nc.sync.dma_start(T[0][:], xq[0])
            nc.sync.dma_start(T[1][:], xq[1])
            nc.sync.dma_start(wlab_sb[:], wlab[:])
            nc.sync.dma_start(rlab_sb[:], rlab[:])
            for i in range(NIT):
                if i + 2 < NB:
                    nc.sync.dma_start(T[i + 2][:], xq[i + 2])
                main_item(i)
                for (gi, j) in bands_after.get(i, ()):
                    band_item(gi, j)

            nc.sync.dma_start(outp[:], stats[:])

    nc.compile()
    return nc


def _prepare_in_maps(X, t):
    perm = np.argsort(t, kind="stable")
    Xs = X[perm]
    ts = t[perm].astype(np.int64)
    counts = np.bincount(ts)
    maxc = int(counts.max()) if counts.size else 0
    assert maxc <= 64, f"class count {maxc} exceeds band half-width 64"
    XT = np.ascontiguousarray(Xs.astype(ml_dtypes.float8_e4m3).T)  # [D, N]
    # [b, p, (g,h), col]
    blocks = np.ascontiguousarray(
        XT.reshape(KT2, 2, 128, NB, B).transpose(3, 2, 0, 1, 4)
    ).reshape(NB, 128, 2 * KT2, B)
    tsf = ts.astype(np.float16)  # exact for ids < 2048

    in_maps = []
    for c in range(NCORES):
        order = [(c + s) % NB for s in range(NB)]
        xqc = np.ascontiguousarray(blocks[order])
        wl = np.empty((128, MT * BW), np.float16)
        rl = np.empty((128, MT), np.float32)
        for gi in range(2):
            base = (c + 8 * gi) % NB
            for j in range(4):
                bj = gi * 4 + j
                r0 = base * B + j * 128
                idx = (np.arange(BW) + r0 - 64) % N
                wl[:, bj * BW:(bj + 1) * BW] = tsf[idx][None, :]
                rl[:, bj] = ts[r0:r0 + 128].astype(np.float32)
        in_maps.append({"xq": xqc, "wlab": wl, "rlab": rl})
    return in_maps


# item weights: diag items (slots 0 and 8 vs themselves) 1x, rest 2x
_W_ITEM = np.array([1.0] + [2.0] * 8 + [1.0] + [2.0] * 7, np.float64)


def _reduce_outputs(results):
    tot = 0.0
    for c in range(NCORES):
        o = np.asarray(results[c]["out"], np.float64)
        r_items = o[:, C_R:C_C].sum(axis=0).reshape(NIT, MH).sum(axis=1)
        c_items = o[:, C_C:C_B].sum(axis=0).reshape(NIT, MH).sum(axis=1)
        neg_items = r_items + MARGIN * c_items
        tot += float((_W_ITEM * neg_items).sum())
        tot += float(o[:, C_B:C_B + MT].sum())
    return np.float32(tot / float(N))


def kernel(inputs, targets, _trace=False, _tmpdir=None):
    X = np.asarray(inputs, dtype=np.float32)
    t = np.asarray(targets)
    assert X.shape == (N, D)

    if "nc" not in _CACHE:
        _CACHE["nc"] = _build_program()
    nc = _CACHE["nc"]

    in_maps = _prepare_in_maps(X, t)
    res = run_bass_kernel_spmd(
        nc, in_maps, list(range(NCORES)), trace=_trace, tmpdir=_tmpdir)
    loss = _reduce_outputs(res.results)
    if _trace:
        return loss, res
    return loss
